# revision 24
# baseline (speedup 1.0000x reference)
"""Trainium2 Bass kernel for nn_BCE_Loss (focal-style BCE-with-logits, mean).

Reference math per anchor row x[0:3] (logits) and integer target c:
    col = 0 if c==1 else 1 if c==3 else 2
    t   = one_hot(col, 3)
    w   = (1-pt)^2,  pt = x*t + (1-x)*(1-t)        [from detached logits]
    bce = max(x,0) - x*t + log1p(exp(-|x|))
    out = mean(w * bce)

Per element this is (x-t)^2 * softplus(v), v = x*(1-2t).  With
g = 0.5 - t in {+-0.5} and h = g*x, two identities remove all per-element
weight math:
    v         = 2*h
    (x - t)^2 = x^2 + 2*h - x - g + 0.5
so the loss sum becomes four dot products against sp = softplus(v) plus
half the plain sum of sp (a fifth FD=1 ones-matmul riding the PE stream,
keeping the sum off the pacing ACT queue):
    S = sum x^2*sp + 2 sum h*sp - sum x*sp - sum g*sp + 0.5 sum sp

Layout: the host ships pred CLASS-PLANAR (per-core [3, n] slab, a pure
relayout of the sharded block) so every on-device tensor is contiguous
blocked [x0|x1|x2] instead of [t,3]-interleaved - elementwise ops do not
care about element order and the mask build loses its 1x-mode strided
writes entirely.  targ ships as bf16 (values 0..4 are exact) so the
one-hot compares run in the DVE's fast single-src modes.

Engine split (per tile, graded sizes 512..1024 rows/partition):
    DVE   g0/g1 compares (~500ns), g2 = 0.5-g0-g1 (stt), h = g*x and
          xsq = x*x (bf16 2x_1p)
    ACT   E = Exp(2h) -> PSUM (keeps the E round-trip off the contended
          SBUF ports) ; sp = Ln(E + 1)
    PE    per 128-chunk: stationary sp_c, moving [xsq|h|x|g] chunk slices
          (FD=512 via a 4-slot mega-tile view, ~216-260ns/chunk)
          accumulating the four diag sums in one PSUM [128,512] region
    x is cast f32->bf16 by the SWDGE cast-DMA directly into the mega
    tile's x slot.

Graded head tiles ([256, 768, 1024*7]) shorten the pipeline fill: the
first softplus tile is ready ~6us earlier, so the PE stream starts early.

Sharding: pure data-parallel across 8 NeuronCores - each core takes a
contiguous block of anchors (per-partition-contiguous within a core);
per-core output is a [128] vector of per-partition partial sums; the
host sums the 8x128 partials and divides by the element count.
"""

import numpy as np

import concourse.bacc as bacc
import concourse.bass as bass
import concourse.mybir as mybir
from concourse import bass_utils
from concourse.alu_op_type import AluOpType
from concourse.tile import TileContext

N_CORES = 8
N_ANCHORS = 8388608
N_CLASSES = 3
N_SHARD = N_ANCHORS // N_CORES  # 1048576
P = 128  # SBUF partitions
A_PART = N_SHARD // P  # 8192 anchors per partition
SIZES = [256, 768, 1024, 1024, 1024, 1024, 1024, 1024, 1024]
assert sum(SIZES) == A_PART
NT = len(SIZES)
MM = 128  # diag-trick matmul chunk width
NG = 4  # PE moving groups: [xsq, h, x, g]


class _Bacc(bacc.Bacc):
    """Bacc with the ACT table pinned to natural_log_exp_and_others.

    The default chooser puts Exp in exp_and_others and Ln in natural_log,
    reloading tables every tile (~2.7us each). Both live in
    natural_log_exp_and_others; emptying every other set (positions kept -
    act_func_set_id is the index into act_info.json) forces one load."""

    _ACT_SET = "natural_log_exp_and_others"

    def insert_act_table_loads(self):
        import bass_rust as _bass_rust

        from concourse.hw_specs import get_activation_tables

        has_activation = any(
            isinstance(i, mybir.InstActivation)
            for b in self.main_func.blocks
            for i in b.instructions
        )
        if not has_activation:
            return
        tables = [
            (name, (fns if name == self._ACT_SET else set()))
            for name, fns in get_activation_tables(self.m.arch).items()
        ]
        _bass_rust.insert_act_table_loads(self, tables)


def _build_nc() -> bass.Bass:
    nc = _Bacc("TRN2", target_bir_lowering=False, num_swdge_queues=4)
    predt = nc.dram_tensor(
        "predt", [N_CLASSES * N_SHARD], mybir.dt.float32, kind="ExternalInput"
    )
    targ = nc.dram_tensor("targb", [N_SHARD], mybir.dt.bfloat16, kind="ExternalInput")
    msgn = nc.dram_tensor("msgn", [P, NG * MM], mybir.dt.bfloat16, kind="ExternalInput")
    out = nc.dram_tensor("out", [P], mybir.dt.float32, kind="ExternalOutput")

    # class-planar: element (j, p, a) -> partition p, block j, col a
    outv = out.rearrange("(p o) -> p o", p=P)
    xvv = predt.rearrange("(j p a) -> p j a", j=N_CLASSES, p=P)
    tvv = targ.rearrange("(p a) -> p a", p=P)

    with TileContext(nc) as tc:
        with (
            tc.tile_pool(name="io", bufs=3) as io,
            tc.tile_pool(name="mega", bufs=3) as megap,
            tc.tile_pool(name="spp", bufs=3) as spp,
            tc.tile_pool(name="singles", bufs=1) as singles,
            tc.tile_pool(name="psum", bufs=1, space="PSUM") as psum,
            tc.tile_pool(name="psE", bufs=1, space="PSUM") as psE,
        ):
            ones_c = singles.tile([P, 1], mybir.dt.bfloat16)
            nc.vector.memset(ones_c, 1.0)
            psA = psum.tile([P, NG * MM], mybir.dt.float32)
            psS = psum.tile([P, 1], mybir.dt.float32)

            n_chunks = sum(3 * s // MM for s in SIZES)
            chunk_id = 0
            off = 0
            for i, size in enumerate(SIZES):
                F = N_CLASSES * size
                # mega-tile slots: 0 = xsq, 1 = h, 2 = x, 3 = g
                B = megap.tile([P, NG * F], mybir.dt.bfloat16)
                B3 = B.rearrange("p (s f) -> p s f", s=NG)
                g = B3[:, 3, :]
                gj = g.rearrange("p (j t) -> p j t", j=N_CLASSES)
                xslot = B3[:, 2, :]

                # x loaded f32->bf16 by the SWDGE cast-DMA into slot 2
                tg = io.tile([P, size], mybir.dt.bfloat16)
                nc.gpsimd.dma_start(
                    out=xslot.rearrange("p (j t) -> p j t", j=N_CLASSES),
                    in_=xvv[:, :, off : off + size])
                nc.sync.dma_start(out=tg, in_=tvv[:, off : off + size])

                # one-hot g planes, all contiguous writes:
                # g0 = (targ != 1) - 0.5 ; g1 = (targ != 3) - 0.5
                nc.vector.tensor_scalar(
                    out=gj[:, 0, :], in0=tg, scalar1=1, scalar2=0.5,
                    op0=AluOpType.not_equal, op1=AluOpType.subtract)
                nc.vector.tensor_scalar(
                    out=gj[:, 1, :], in0=tg, scalar1=3, scalar2=0.5,
                    op0=AluOpType.not_equal, op1=AluOpType.subtract)
                # g2 = 0.5 - g0 - g1  (reverse0: scalar - in0)
                ic2 = nc.vector.scalar_tensor_tensor(
                    out=gj[:, 2, :], in0=gj[:, 0, :], scalar=0.5,
                    in1=gj[:, 1, :],
                    op0=AluOpType.subtract, op1=AluOpType.subtract)
                ic2.ins.reverse0 = True

                # h = g*x ; xsq = x*x
                nc.vector.tensor_tensor(
                    out=B3[:, 1, :], in0=g, in1=xslot, op=AluOpType.mult)
                nc.vector.tensor_tensor(
                    out=B3[:, 0, :], in0=xslot, in1=xslot, op=AluOpType.mult)

                # E = exp(2h) = e^v, PSUM-resident (keeps the E round-trip
                # off the contended SBUF ports) ; sp = ln(E+1) = softplus(v)
                E = psE.tile([P, F], mybir.dt.float32)
                nc.scalar.activation(
                    out=E, in_=B3[:, 1, :],
                    func=mybir.ActivationFunctionType.Exp, scale=2.0)
                sp = spp.tile([P, F], mybir.dt.bfloat16)
                nc.scalar.activation(
                    out=sp, in_=E, func=mybir.ActivationFunctionType.Ln,
                    bias=1.0)

                # PE: psA += sp_c^T @ [xsq_c | h_c | x_c | g_c]; the four
                # 128-col group diagonals accumulate the four dot products
                for c in range(F // MM):
                    s = slice(c * MM, (c + 1) * MM)
                    nc.tensor.matmul(
                        psA[:, :], sp[:, s], B3[:, :, s],
                        start=(chunk_id == 0),
                        stop=(chunk_id == n_chunks - 1))
                    # sum sp rides the PE too: psS[i,0] += sum_p sp[p, c+i]
                    nc.tensor.matmul(
                        psS[:, :], sp[:, s], ones_c[:, :],
                        start=(chunk_id == 0),
                        stop=(chunk_id == n_chunks - 1))
                    chunk_id += 1
                off += size

            # epilogue: S = (+1,+2,-1,-1) . group diags + 0.5 * sum accsp
            msgn_t = singles.tile([P, NG * MM], mybir.dt.bfloat16)
            nc.sync.dma_start(out=msgn_t, in_=msgn[:, :])
            dm = singles.tile([P, NG * MM], mybir.dt.float32)
            nc.vector.tensor_tensor(out=dm, in0=psA, in1=msgn_t, op=AluOpType.mult)
            r1 = singles.tile([P, 1], mybir.dt.float32)
            nc.vector.tensor_reduce(
                out=r1, in_=dm, axis=mybir.AxisListType.X, op=AluOpType.add)
            tot = singles.tile([P, 1], mybir.dt.float32)
            nc.vector.scalar_tensor_tensor(
                out=tot, in0=psS, scalar=0.5, in1=r1,
                op0=AluOpType.mult, op1=AluOpType.add)
            nc.sync.dma_start(out=outv, in_=tot[:, :])

    nc.compile()
    return nc


_cache: dict[str, bass.Bass] = {}
last_results = None  # BassKernelResults of the most recent run (for test.py)


def _get_nc() -> bass.Bass:
    if "nc" not in _cache:
        _cache["nc"] = _build_nc()
    return _cache["nc"]


def _msgn_bf16() -> np.ndarray:
    import ml_dtypes

    coefs = [1.0, 2.0, -1.0, -1.0]  # xsq, h, x, g
    m = np.zeros((P, NG * MM), dtype=np.float32)
    idx = np.arange(P)
    for s, cf in enumerate(coefs):
        m[idx, s * MM + idx] = cf
    return m.astype(ml_dtypes.bfloat16)


def kernel(pred: np.ndarray, targ: np.ndarray, *, trace: bool = False) -> np.ndarray:
    global last_results
    import ml_dtypes

    pred = np.ascontiguousarray(np.asarray(pred, dtype=np.float32))
    targ = np.asarray(targ)
    assert pred.shape == (N_ANCHORS, N_CLASSES), pred.shape
    assert targ.shape == (N_ANCHORS,), targ.shape

    # lossless bf16 image of the index tensor (values 0..4 are exact)
    targb = np.ascontiguousarray(targ.astype(ml_dtypes.bfloat16))

    nc = _get_nc()
    msgn = _msgn_bf16()

    in_maps = []
    for c in range(N_CORES):
        sl = slice(c * N_SHARD, (c + 1) * N_SHARD)
        # per-core class-planar relayout of the sharded block
        predt = np.ascontiguousarray(pred[sl].T).reshape(-1)
        in_maps.append({
            "predt": predt,
            "targb": targb[sl],
            "msgn": msgn,
        })

    res = bass_utils.run_bass_kernel_spmd(
        nc, in_maps, core_ids=list(range(N_CORES)), trace=trace
    )
    last_results = res

    total = np.float64(0.0)
    for r in res.results:
        total += np.float64(r["out"]).sum()
    mean = total / (N_ANCHORS * N_CLASSES)
    return np.float32(mean)


# revision 25
# speedup vs baseline: 1.0675x; 1.0675x over previous
"""Trainium2 Bass kernel for nn_BCE_Loss (focal-style BCE-with-logits, mean).

Reference math per anchor row x[0:3] (logits) and integer target c:
    col = 0 if c==1 else 1 if c==3 else 2
    t   = one_hot(col, 3)
    w   = (1-pt)^2,  pt = x*t + (1-x)*(1-t)        [from detached logits]
    bce = max(x,0) - x*t + log1p(exp(-|x|))
    out = mean(w * bce)

Per element this is (x-t)^2 * softplus(v), v = x*(1-2t).  With
g = 0.5 - t in {+-0.5} and h = g*x, two identities remove all per-element
weight math:
    v         = 2*h
    (x - t)^2 = x^2 + 2*h - x - g + 0.5
so the loss sum becomes four dot products against sp = softplus(v) plus
half the plain sum of sp (delivered for free by the Ln op's accum_out):
    S = sum x^2*sp + 2 sum h*sp - sum x*sp - sum g*sp + 0.5 sum sp

Layout: the host ships pred CLASS-PLANAR (per-core [3, n] slab, a pure
relayout of the sharded block) so every on-device tensor is contiguous
blocked [x0|x1|x2] instead of [t,3]-interleaved - elementwise ops do not
care about element order and the mask build loses its 1x-mode strided
writes entirely.  targ ships as bf16 (values 0..4 are exact) so the
one-hot compares run in the DVE's fast single-src modes.

Engine split (per tile, graded sizes 512..1024 rows/partition):
    DVE   g0/g1 compares (~500ns), g2 = 0.5-g0-g1 (stt), h = g*x and
          xsq = x*x (bf16 2x_1p)
    ACT   E = Exp(2h) -> PSUM (keeps the E round-trip off the contended
          SBUF ports) ; sp = Ln(E + 1) with accum_out = per-partition
          sum of sp
    PE    per 128-chunk: stationary sp_c, moving [xsq|h|x|g] chunk slices
          (FD=512 via a 4-slot mega-tile view, ~216-260ns/chunk)
          accumulating the four diag sums in one PSUM [128,512] region
    x is cast f32->bf16 by the SWDGE cast-DMA directly into the mega
    tile's x slot.

Graded head tiles ([256, 768, 1024*7]) shorten the pipeline fill: the
first softplus tile is ready ~6us earlier, so the PE stream starts early.

Sharding: pure data-parallel across 8 NeuronCores - each core takes a
contiguous block of anchors (per-partition-contiguous within a core);
per-core output is a [128] vector of per-partition partial sums; the
host sums the 8x128 partials and divides by the element count.
"""

import numpy as np

import concourse.bacc as bacc
import concourse.bass as bass
import concourse.mybir as mybir
from concourse import bass_utils
from concourse.alu_op_type import AluOpType
from concourse.tile import TileContext

N_CORES = 8
N_ANCHORS = 8388608
N_CLASSES = 3
N_SHARD = N_ANCHORS // N_CORES  # 1048576
P = 128  # SBUF partitions
A_PART = N_SHARD // P  # 8192 anchors per partition
SIZES = [256, 768, 1024, 1024, 1024, 1024, 1024, 1024, 1024]
assert sum(SIZES) == A_PART
NT = len(SIZES)
MM = 128  # diag-trick matmul chunk width
NG = 4  # PE moving groups: [xsq, h, x, g]


class _Bacc(bacc.Bacc):
    """Bacc with the ACT table pinned to natural_log_exp_and_others.

    The default chooser puts Exp in exp_and_others and Ln in natural_log,
    reloading tables every tile (~2.7us each). Both live in
    natural_log_exp_and_others; emptying every other set (positions kept -
    act_func_set_id is the index into act_info.json) forces one load."""

    _ACT_SET = "natural_log_exp_and_others"

    def insert_act_table_loads(self):
        import bass_rust as _bass_rust

        from concourse.hw_specs import get_activation_tables

        has_activation = any(
            isinstance(i, mybir.InstActivation)
            for b in self.main_func.blocks
            for i in b.instructions
        )
        if not has_activation:
            return
        tables = [
            (name, (fns if name == self._ACT_SET else set()))
            for name, fns in get_activation_tables(self.m.arch).items()
        ]
        _bass_rust.insert_act_table_loads(self, tables)


def _build_nc() -> bass.Bass:
    nc = _Bacc("TRN2", target_bir_lowering=False, num_swdge_queues=4)
    predt = nc.dram_tensor(
        "predt", [N_CLASSES * N_SHARD], mybir.dt.float32, kind="ExternalInput"
    )
    targ = nc.dram_tensor("targb", [N_SHARD], mybir.dt.bfloat16, kind="ExternalInput")
    msgn = nc.dram_tensor("msgn", [P, NG * MM], mybir.dt.bfloat16, kind="ExternalInput")
    out = nc.dram_tensor("out", [P], mybir.dt.float32, kind="ExternalOutput")

    outv = out.rearrange("(p o) -> p o", p=P)
    # class-planar: element (j, p, a) -> partition p, block j, col a
    xvv = predt.rearrange("(j p a) -> p j a", j=N_CLASSES, p=P)
    tvv = targ.rearrange("(p a) -> p a", p=P)

    with TileContext(nc) as tc:
        with (
            tc.tile_pool(name="io", bufs=3) as io,
            tc.tile_pool(name="mega", bufs=3) as megap,
            tc.tile_pool(name="spp", bufs=3) as spp,
            tc.tile_pool(name="singles", bufs=1) as singles,
            tc.tile_pool(name="psum", bufs=1, space="PSUM") as psum,
            tc.tile_pool(name="psE", bufs=1, space="PSUM") as psE,
        ):
            accsp = singles.tile([P, NT], mybir.dt.float32)
            psA = psum.tile([P, NG * MM], mybir.dt.float32)

            n_chunks = sum(3 * s // MM for s in SIZES)
            chunk_id = 0
            off = 0
            for i, size in enumerate(SIZES):
                F = N_CLASSES * size
                # mega-tile slots: 0 = xsq, 1 = h, 2 = x, 3 = g
                B = megap.tile([P, NG * F], mybir.dt.bfloat16)
                B3 = B.rearrange("p (s f) -> p s f", s=NG)
                g = B3[:, 3, :]
                gj = g.rearrange("p (j t) -> p j t", j=N_CLASSES)
                xslot = B3[:, 2, :]

                # x loaded f32->bf16 by the SWDGE cast-DMA into slot 2
                tg = io.tile([P, size], mybir.dt.bfloat16)
                nc.gpsimd.dma_start(
                    out=xslot.rearrange("p (j t) -> p j t", j=N_CLASSES),
                    in_=xvv[:, :, off : off + size])
                nc.sync.dma_start(out=tg, in_=tvv[:, off : off + size])

                # one-hot g planes, all contiguous writes:
                # g0 = (targ != 1) - 0.5 ; g1 = (targ != 3) - 0.5
                nc.vector.tensor_scalar(
                    out=gj[:, 0, :], in0=tg, scalar1=1, scalar2=0.5,
                    op0=AluOpType.not_equal, op1=AluOpType.subtract)
                nc.vector.tensor_scalar(
                    out=gj[:, 1, :], in0=tg, scalar1=3, scalar2=0.5,
                    op0=AluOpType.not_equal, op1=AluOpType.subtract)
                # g2 = 0.5 - g0 - g1  (reverse0: scalar - in0)
                ic2 = nc.vector.scalar_tensor_tensor(
                    out=gj[:, 2, :], in0=gj[:, 0, :], scalar=0.5,
                    in1=gj[:, 1, :],
                    op0=AluOpType.subtract, op1=AluOpType.subtract)
                ic2.ins.reverse0 = True

                # h = g*x ; xsq = x*x
                nc.vector.tensor_tensor(
                    out=B3[:, 1, :], in0=g, in1=xslot, op=AluOpType.mult)
                nc.vector.tensor_tensor(
                    out=B3[:, 0, :], in0=xslot, in1=xslot, op=AluOpType.mult)

                # E = exp(2h) = e^v, PSUM-resident (keeps the E round-trip
                # off the contended SBUF ports) ; sp = ln(E+1) = softplus(v)
                E = psE.tile([P, F], mybir.dt.float32)
                nc.scalar.activation(
                    out=E, in_=B3[:, 1, :],
                    func=mybir.ActivationFunctionType.Exp, scale=2.0)
                sp = spp.tile([P, F], mybir.dt.bfloat16)
                nc.scalar.activation(
                    out=sp, in_=E, func=mybir.ActivationFunctionType.Ln,
                    bias=1.0, accum_out=accsp[:, i : i + 1])

                # PE: psA += sp_c^T @ [xsq_c | h_c | x_c | g_c]; the four
                # 128-col group diagonals accumulate the four dot products
                for c in range(F // MM):
                    s = slice(c * MM, (c + 1) * MM)
                    nc.tensor.matmul(
                        psA[:, :], sp[:, s], B3[:, :, s],
                        start=(chunk_id == 0),
                        stop=(chunk_id == n_chunks - 1))
                    chunk_id += 1
                off += size

            # epilogue: S = (+1,+2,-1,-1) . group diags + 0.5 * sum accsp
            msgn_t = singles.tile([P, NG * MM], mybir.dt.bfloat16)
            nc.sync.dma_start(out=msgn_t, in_=msgn[:, :])
            dm = singles.tile([P, NG * MM], mybir.dt.float32)
            nc.vector.tensor_tensor(out=dm, in0=psA, in1=msgn_t, op=AluOpType.mult)
            r1 = singles.tile([P, 1], mybir.dt.float32)
            nc.vector.tensor_reduce(
                out=r1, in_=dm, axis=mybir.AxisListType.X, op=AluOpType.add)
            racc = singles.tile([P, 1], mybir.dt.float32)
            nc.vector.tensor_reduce(
                out=racc, in_=accsp, axis=mybir.AxisListType.X, op=AluOpType.add)
            tot = singles.tile([P, 1], mybir.dt.float32)
            nc.vector.scalar_tensor_tensor(
                out=tot, in0=racc, scalar=0.5, in1=r1,
                op0=AluOpType.mult, op1=AluOpType.add)
            nc.sync.dma_start(out=outv, in_=tot[:, :])

    nc.compile()
    return nc


_cache: dict[str, bass.Bass] = {}
last_results = None  # BassKernelResults of the most recent run (for test.py)


def _get_nc() -> bass.Bass:
    if "nc" not in _cache:
        _cache["nc"] = _build_nc()
    return _cache["nc"]


def _msgn_bf16() -> np.ndarray:
    import ml_dtypes

    coefs = [1.0, 2.0, -1.0, -1.0]  # xsq, h, x, g
    m = np.zeros((P, NG * MM), dtype=np.float32)
    idx = np.arange(P)
    for s, cf in enumerate(coefs):
        m[idx, s * MM + idx] = cf
    return m.astype(ml_dtypes.bfloat16)


def kernel(pred: np.ndarray, targ: np.ndarray, *, trace: bool = False) -> np.ndarray:
    global last_results
    import ml_dtypes

    pred = np.ascontiguousarray(np.asarray(pred, dtype=np.float32))
    targ = np.asarray(targ)
    assert pred.shape == (N_ANCHORS, N_CLASSES), pred.shape
    assert targ.shape == (N_ANCHORS,), targ.shape

    # lossless bf16 image of the index tensor (values 0..4 are exact)
    targb = np.ascontiguousarray(targ.astype(ml_dtypes.bfloat16))

    nc = _get_nc()
    msgn = _msgn_bf16()

    in_maps = []
    for c in range(N_CORES):
        sl = slice(c * N_SHARD, (c + 1) * N_SHARD)
        # per-core class-planar relayout of the sharded block
        predt = np.ascontiguousarray(pred[sl].T).reshape(-1)
        in_maps.append({
            "predt": predt,
            "targb": targb[sl],
            "msgn": msgn,
        })

    res = bass_utils.run_bass_kernel_spmd(
        nc, in_maps, core_ids=list(range(N_CORES)), trace=trace
    )
    last_results = res

    total = np.float64(0.0)
    for r in res.results:
        total += np.float64(r["out"]).sum()
    mean = total / (N_ANCHORS * N_CLASSES)
    return np.float32(mean)
